# revision 68
# baseline (speedup 1.0000x reference)
"""Trainium2 Bass kernel for nn_ChannelCompressAttention.

Shapes: x (8, 4096, 1024) f32, w_qkv (3072, 1024) f32, w_conv1 (1024,) f32.
Output: (8, 4096, 1024) f32.

Math (all q/k/v uses are rank-1 contractions):
  u  = scale * Wq^T w_conv1      agent = x u           (per batch)
  s  = x^T agent                 z     = Wk s
  aa = softmax(z)                t     = Wv^T aa
  sc = x t                       p     = softmax(sc)
  r  = x^T p                     out_row = Wv r
Every output row equals out_row (the trailing singleton softmax == 1), so
the device ships one (128, 8) f32 column block + the pass-2 partition
function Z2; the host normalizes, re-orders and broadcasts to (n, c).

Softmax handling (shortens the serial mid-chains):
  - aa's normalizer 1/Z1 is NOT applied to t. Instead pass-2 computes
    p ∝ exp(sc~ / Z1) by passing scale=1/Z1 (a per-partition AP) to the
    ScalarE Exp. The 1/Z1 reciprocal+broadcast runs off the critical
    path, in parallel with the t matmuls.
  - p's normalizer 1/Z2 is shipped to the host (one f32), so r/vo stay
    unnormalized on device. Logits are O(30) (f32/bf16 headroom to
    ~e38), so no max subtraction anywhere.

On-core mapping (x resident in SBUF, natural (n-part, c-free) layout):
  - c-contractions (agent, sc, z, vo): DVE/ACT lane-balanced: per
    (128,1024) bf16 tile either one fused DVE STT (1x, ~1.2us) or DVE
    TT multiply (2x, ~0.69us) + ScalarE accum-copy (~1.15us). A ~1:2
    STT:TT split equalizes the two engine queues.
  - n-contractions (s, r) and t: TensorE rank-1 row form (lhsT (128,1)
    bf16, rhs (128,512) halves) accumulated in PSUM.
  - tiles are processed in groups of 4 (one DMA chunk): per group one
    DVE copy casts the 4 f32 dot accums to bf16 lhsT columns, one
    (128,4) ScalarE Exp in pass 2. Group finishers trail the dot
    stream by 2 groups so neither engine head-of-line blocks.
  - dummy (128,512) matmuls during the z-window keep the PE's HAM
    clock at 8/8 so the t matmuls run warm.

DMA: x ships as 8x 1MiB chunks (95%+ of the 358 GB/s per-core HBM
roof), then Wk, then Wv (each 2x 1MiB) - the exact consumption order.
u is a 2KiB row, broadcast on-device via the ones-matmul (also warms
the PE before pass 1).

Sharding: data-parallel over batch, one batch per NeuronCore (8 cores).
"""

import sys

for _p in ("/opt/trn_rl_repo", "/opt/pypackages"):
    if _p not in sys.path:
        sys.path.insert(0, _p)

import numpy as np
import ml_dtypes

import concourse.bacc as bacc
import concourse.mybir as mybir
import concourse.tile as tile
from concourse.bass_utils import run_bass_kernel_spmd

B, N, C = 8, 4096, 1024
P = 128
NT = N // P          # 32 x-tiles per batch
J = C // P           # 8 weight tiles per matrix
G = 4                # tiles per group == tiles per DMA chunk
NG = NT // G         # 8 groups
F32 = mybir.dt.float32
BF16 = mybir.dt.bfloat16
NPBF = ml_dtypes.bfloat16
SCALE = float(C) ** -0.5
H = 512
XF = NT * C          # 32768 free elements in the big x row
WF = J * C           # 8192 free elements per weight matrix


def _build():
    nc = bacc.Bacc(None)
    xb = nc.declare_dram_parameter("xb", [P, XF], BF16, isOutput=False)
    wk = nc.declare_dram_parameter("wk", [P, WF], BF16, isOutput=False)
    wv = nc.declare_dram_parameter("wv", [P, WF], BF16, isOutput=False)
    ubc = nc.declare_dram_parameter("ubc", [P, C], BF16, isOutput=False)
    out = nc.declare_dram_parameter("out", [P, J], F32, isOutput=True)
    z2o = nc.declare_dram_parameter("z2o", [1, 1], F32, isOutput=True)

    mult = mybir.AluOpType.mult
    add = mybir.AluOpType.add
    AF = mybir.ActivationFunctionType

    with tile.TileContext(nc) as tc:
        with (
            tc.tile_pool(name="xres", bufs=8) as xpool,
            tc.tile_pool(name="wres", bufs=2) as wpool,
            tc.tile_pool(name="bc", bufs=3) as bcpool,
            tc.tile_pool(name="scrd", bufs=6) as scrd,
            tc.tile_pool(name="cols", bufs=1) as cols,
            tc.tile_pool(name="rows", bufs=3) as rows,
            tc.tile_pool(name="small", bufs=1) as small,
            tc.tile_pool(name="ps", bufs=7, space="PSUM") as psp,
            tc.tile_pool(name="psw", bufs=1, space="PSUM") as psw,
        ):
            ones_m = small.tile([1, P], BF16, tag="ones_m")  # lhsT: row bcast
            nc.vector.memset(ones_m, 1.0)
            ones_k = small.tile([P, 1], F32, tag="ones_k")   # rhs: part sum
            nc.vector.memset(ones_k, 1.0)

            # u arrives pre-broadcast from host (256 KiB, first in queue)
            u_bc = small.tile([P, C], BF16, tag="ubc")
            nc.sync.dma_start(out=u_bc, in_=ubc[:, :])

            # x stream: 8 chunks of (128, 4096) = 1 MiB each, then weights
            NCH = 8
            xch = []
            for q in range(NCH):
                t = xpool.tile([P, XF // NCH], BF16, tag="x")
                nc.sync.dma_start(out=t, in_=xb[:, q * (XF // NCH):(q + 1) * (XF // NCH)])
                xch.append(t)
            wkt = []
            for q in range(2):
                t = wpool.tile([P, WF // 2], BF16, tag="wk")
                nc.sync.dma_start(out=t, in_=wk[:, q * (WF // 2):(q + 1) * (WF // 2)])
                wkt.append(t)
            wvt = []
            for q in range(2):
                t = wpool.tile([P, WF // 2], BF16, tag="wv")
                nc.sync.dma_start(out=t, in_=wv[:, q * (WF // 2):(q + 1) * (WF // 2)])
                wvt.append(t)

            def xtile(i):
                return xch[i // G][:, (i % G) * C:(i % G) * C + C]

            def wtile(ts, j):
                return ts[j // 4][:, (j % 4) * C:(j % 4) * C + C]

            act_dummy = small.tile([P, C], BF16, tag="actd")

            def cdot(kind, xt, bc, acc):
                # acc[p] = sum_c xt[p,c]*bc[p,c] on one of two lanes
                if kind:  # DVE TT multiply (2x) + ScalarE accum copy (1x)
                    prod = scrd.tile([P, C], BF16, tag="prod")
                    nc.vector.tensor_tensor(out=prod, in0=xt, in1=bc, op=mult)
                    nc.scalar.activation(out=act_dummy, in_=prod,
                                         func=AF.Copy, accum_out=acc)
                else:     # fused DVE STT (1x)
                    scr = scrd.tile([P, C], BF16, tag="scr")
                    nc.vector.scalar_tensor_tensor(
                        out=scr, in0=xt, scalar=1.0, in1=bc,
                        op0=mult, op1=mult, accum_out=acc)

            def acc_pair(nm):
                lo = psp.tile([1, H], F32, tag="ps", name=f"{nm}_lo")
                hi = psp.tile([1, H], F32, tag="ps", name=f"{nm}_hi")
                return lo, hi

            def psum_to_row(ps_lo, ps_hi):
                row = rows.tile([1, C], BF16, tag="row")
                nc.scalar.activation(out=row[:, 0:H], in_=ps_lo, func=AF.Copy)
                nc.vector.tensor_copy(out=row[:, H:C], in_=ps_hi)
                return row

            def bcast_row(row):
                dest = bcpool.tile([P, C], BF16, tag="bc")
                ps0 = psp.tile([P, H], F32, tag="ps")
                nc.tensor.matmul(ps0, lhsT=ones_m, rhs=row[:, 0:H],
                                 start=True, stop=True)
                ps1 = psp.tile([P, H], F32, tag="ps")
                nc.tensor.matmul(ps1, lhsT=ones_m, rhs=row[:, H:C],
                                 start=True, stop=True)
                nc.scalar.activation(out=dest[:, 0:H], in_=ps0, func=AF.Copy)
                nc.vector.tensor_copy(out=dest[:, H:C], in_=ps1)
                return dest

            # HAM keep-warm: the PE re-throttles to 1.2 GHz after any
            # ~3.4us idle window, which measured as ~50% of all matmuls
            # running at half clock. One junk (1,512) matmul per tile-dot
            # (lhsT = an already-computed column, so issue tracks real
            # progress) denies the idle window its trigger.
            warm1 = psw.tile([1, H], F32, tag="warm1")

            def keep_warm(col):
                nc.tensor.matmul(warm1, lhsT=col, rhs=u_bc[:, 0:H],
                                 start=True, stop=True)

            # STT (fused, 1x) on 12 of 32 tiles balances DVE vs ACT queues
            # (measured ACT 83.3us vs DVE 79.3us busy at the 11:21 split).
            # Pass 2 carries the exps on ACT, so one more tile goes STT.
            def lane(i):
                return i % 8 not in (1, 4, 6)

            def lane2(i):
                return i % 8 not in (1, 4, 6) and i != 30

            # ---- pass 1: agent_i = x_i u (DVE/ACT); s += x_i^T agent_i (PE)
            agf = cols.tile([P, NT], F32, tag="agf")
            agb = cols.tile([P, NT], BF16, tag="agb")
            s_lo, s_hi = acc_pair("s")

            GROUPS = [(k * G, G) for k in range(NG)]

            def p1_finish(gi):
                lo, sz = GROUPS[gi]
                hi = lo + sz
                nc.vector.tensor_copy(out=agb[:, lo:hi], in_=agf[:, lo:hi])
                for i in range(lo, hi):
                    nc.tensor.matmul(s_lo, lhsT=agb[:, i:i + 1],
                                     rhs=xtile(i)[:, 0:H],
                                     start=(i == 0), stop=(i == NT - 1))
                    nc.tensor.matmul(s_hi, lhsT=agb[:, i:i + 1],
                                     rhs=xtile(i)[:, H:C],
                                     start=(i == 0), stop=(i == NT - 1))

            for gi, (lo, sz) in enumerate(GROUPS):
                if gi >= 2:
                    p1_finish(gi - 2)
                for i in range(lo, lo + sz):
                    cdot(lane(i), xtile(i), u_bc, agf[:, i:i + 1])
            p1_finish(len(GROUPS) - 2)
            p1_finish(len(GROUPS) - 1)

            s_bc = bcast_row(psum_to_row(s_lo, s_hi))

            # ---- z_j = Wk_j s (DVE/ACT); ez_j = exp(z_j) (ACT);
            #      t += ez_j^T Wv_j (PE); dummy MMs keep the PE warm ----
            zf = cols.tile([P, J], F32, tag="zf")
            ez = cols.tile([P, J], BF16, tag="ez")
            t_lo, t_hi = acc_pair("t")
            for j in range(J):
                cdot(j % 2 == 0, wtile(wkt, j), s_bc, zf[:, j:j + 1])
                nc.scalar.activation(out=ez[:, j:j + 1], in_=zf[:, j:j + 1],
                                     func=AF.Exp)
                keep_warm(ez[:, j:j + 1])
                nc.tensor.matmul(t_lo, lhsT=ez[:, j:j + 1],
                                 rhs=wtile(wvt, j)[:, 0:H],
                                 start=(j == 0), stop=(j == J - 1))
                nc.tensor.matmul(t_hi, lhsT=ez[:, j:j + 1],
                                 rhs=wtile(wvt, j)[:, H:C],
                                 start=(j == 0), stop=(j == J - 1))

            # 1/Z1 -> (128,1) bcast, off the critical path (t matmuls run
            # meanwhile); consumed by pass-2's Exp as its scale operand.
            ez_rs = small.tile([P, 1], F32, tag="ezrs")
            nc.vector.tensor_reduce(out=ez_rs, in_=ez,
                                    axis=mybir.AxisListType.X, op=add)
            z1ps = psp.tile([1, 1], F32, tag="ps")
            nc.tensor.matmul(z1ps, lhsT=ez_rs, rhs=ones_k, start=True,
                             stop=True)
            rz1 = small.tile([1, 1], F32, tag="rz1")
            nc.vector.reciprocal(out=rz1, in_=z1ps)
            rz1b = small.tile([1, 1], BF16, tag="rz1b")
            nc.vector.tensor_copy(out=rz1b, in_=rz1)
            rz1ps = psp.tile([P, 1], F32, tag="ps")
            nc.tensor.matmul(rz1ps, lhsT=ones_m, rhs=rz1b, start=True,
                             stop=True)
            rz1_bc = small.tile([P, 1], F32, tag="rz1bc")
            nc.vector.tensor_copy(out=rz1_bc, in_=rz1ps)

            t_bc = bcast_row(psum_to_row(t_lo, t_hi))

            # ---- pass 2: sc_i = x_i t~ (DVE/ACT); ep = exp(sc/Z1) per
            #      group (one (128,4) ScalarE Exp); r += x_i^T ep_i (PE) ----
            scf = cols.tile([P, NT], F32, tag="scf")
            epb = cols.tile([P, NT], BF16, tag="epb")
            r_lo, r_hi = acc_pair("r")

            def p2_finish(gi):
                lo, sz = GROUPS[gi]
                hi = lo + sz
                nc.scalar.activation(out=epb[:, lo:hi], in_=scf[:, lo:hi],
                                     func=AF.Exp, scale=rz1_bc)
                for i in range(lo, hi):
                    nc.tensor.matmul(r_lo, lhsT=epb[:, i:i + 1],
                                     rhs=xtile(i)[:, 0:H],
                                     start=(i == 0), stop=(i == NT - 1))
                    nc.tensor.matmul(r_hi, lhsT=epb[:, i:i + 1],
                                     rhs=xtile(i)[:, H:C],
                                     start=(i == 0), stop=(i == NT - 1))

            for gi, (lo, sz) in enumerate(GROUPS):
                if gi >= 2:
                    p2_finish(gi - 2)
                for i in range(lo, lo + sz):
                    cdot(lane2(i), xtile(i), t_bc, scf[:, i:i + 1])
            p2_finish(len(GROUPS) - 2)
            p2_finish(len(GROUPS) - 1)

            # Z2 (unnormalized) ships to the host
            ep_rs = small.tile([P, 1], F32, tag="eprs")
            nc.vector.tensor_reduce(out=ep_rs, in_=epb,
                                    axis=mybir.AxisListType.X, op=add)
            z2ps = psp.tile([1, 1], F32, tag="ps")
            nc.tensor.matmul(z2ps, lhsT=ep_rs, rhs=ones_k, start=True,
                             stop=True)
            z2sb = small.tile([1, 1], F32, tag="z2sb")
            nc.vector.tensor_copy(out=z2sb, in_=z2ps)
            nc.sync.dma_start(out=z2o[:, :], in_=z2sb)

            r_bc = bcast_row(psum_to_row(r_lo, r_hi))

            # ---- vo[:, j] = (Wv r~)[j*128 + p]; host normalizes/re-orders
            # 4:4 split (3:5 measured worse twice despite makespan theory)
            vo_col = small.tile([P, J], F32, tag="vo")
            for j in range(J):
                cdot(j % 2 == 0, wtile(wvt, j), r_bc, vo_col[:, j:j + 1])
            nc.sync.dma_start(out=out[:, :], in_=vo_col)

    return nc


_CACHE = {}


def _get_nc():
    if "nc" not in _CACHE:
        nc = _build()
        nc.finalize()
        _CACHE["nc"] = nc
    return _CACHE["nc"]


def _bigrow(m):
    # (R*128, C) -> (128, R*C): big[p, i*C + c] = m[i*128 + p, c]
    r = m.shape[0] // P
    return np.ascontiguousarray(
        m.reshape(r, P, m.shape[1]).transpose(1, 0, 2).reshape(P, r * m.shape[1]))


def _prep(x, w_qkv, w_conv1):
    x = np.asarray(x, dtype=np.float32)
    w_qkv = np.asarray(w_qkv, dtype=np.float32)
    w_conv1 = np.asarray(w_conv1, dtype=np.float32)
    wq, wkm, wvm = w_qkv[:C], w_qkv[C:2 * C], w_qkv[2 * C:]
    u = (SCALE * (wq.T.astype(np.float64)
                  @ w_conv1.astype(np.float64))).astype(np.float32)
    ubc = np.ascontiguousarray(
        np.broadcast_to(u.astype(NPBF), (P, C)))
    wk_b = _bigrow(wkm.astype(NPBF))
    wv_b = _bigrow(wvm.astype(NPBF))
    xb = np.stack([_bigrow(x[b].astype(NPBF)) for b in range(B)])
    return xb, wk_b, wv_b, ubc


def run(x, w_qkv, w_conv1, **spmd_kwargs):
    xb, wk_b, wv_b, ubc = _prep(x, w_qkv, w_conv1)
    in_maps = [{"xb": xb[b], "wk": wk_b, "wv": wv_b, "ubc": ubc}
               for b in range(B)]
    res = run_bass_kernel_spmd(_get_nc(), in_maps, list(range(B)),
                               **spmd_kwargs)
    out = np.empty((B, N, C), dtype=np.float32)
    for b in range(B):
        z2 = float(res.results[b]["z2o"][0, 0])
        row = res.results[b]["out"].T.reshape(C) / z2
        out[b] = row[None, :]
    return out, res


def kernel(x, w_qkv, w_conv1):
    out, _ = run(x, w_qkv, w_conv1)
    return out
